# revision 1
# baseline (speedup 1.0000x reference)
"""Trainium2 Bass kernel for BaichuanAttention (hidden=5120, 40 heads, b=2, s=2048).

Tensor-parallel over heads across 8 NeuronCores: each core computes QKV for its
5 heads (sharded W_pack rows), flash-style causal attention, and a partial
o_proj (sharded W_o columns); partials are combined with an on-device
ReduceScatter and reassembled on the host.

Matmuls run as float32r (TF32-like fast fp32 path, ~1.5e-4 rel err).
"""

import math
import sys

for _p in ("/opt/trn_rl_repo",):
    if _p not in sys.path:
        sys.path.insert(0, _p)

import numpy as np

import concourse.bass as bass
import concourse.mybir as mybir
import concourse.tile as tile
from concourse import bacc, bass_utils

F32 = mybir.dt.float32
F32R = mybir.dt.float32r


class Cfg:
    def __init__(self, hidden=5120, n_heads=40, dh=128, B=2, S=2048, n_cores=8):
        self.hidden = hidden
        self.n_heads = n_heads
        self.dh = dh
        self.B = B
        self.S = S
        self.n_cores = n_cores
        assert dh == 128
        self.HL = n_heads // n_cores          # heads per core
        self.F = 3 * self.HL * dh             # per-core packed qkv rows
        self.FO = self.HL * dh                # per-core o_proj input width
        self.T = B * S                        # total tokens
        self.KT = hidden // 128               # contraction tiles for qkv
        self.TC = self.T // 512               # token chunks for qkv
        self.SQT = S // 128                   # q tiles per batch
        self.QC = S // 512                    # q chunks per batch
        self.OC = hidden // 512               # o chunks
        self.FTO = self.FO // 128             # attn feature tiles per core
        self.TG = max(1, self.T // 1024)      # reduce-scatter groups
        assert self.T % (self.TG * n_cores) == 0
        self.RS_ROWS = self.T // (self.TG * n_cores)  # out rows per core per group

    def key(self):
        return (self.hidden, self.n_heads, self.dh, self.B, self.S, self.n_cores)


def _ft_splits(n, cap=8):
    out = []
    while n > 0:
        take = min(cap, n)
        # avoid a tiny trailing pass
        if 0 < n - take < 3 and take > 4:
            take = n - 2
        out.append(take)
        n -= take
    return out


def build_program(cfg: Cfg, mode: str, phases: str = "ABC"):
    """mode: 'causal' (ignore mask input, causal skip), 'dense' (zero mask),
    'masked' (general additive mask input, pre-scaled by sqrt(dh) on host)."""
    assert mode in ("causal", "dense", "masked")
    c = cfg
    nc = bacc.Bacc("TRN2", target_bir_lowering=False, debug=False,
                   num_devices=c.n_cores)
    mask_ext = None
    xt = nc.dram_tensor("xt", [c.hidden, c.T], F32R, kind="ExternalInput").ap()
    wqkvt = nc.dram_tensor("wqkvt", [c.hidden, c.F], F32R,
                           kind="ExternalInput").ap()
    wot = nc.dram_tensor("wot", [c.FO, c.hidden], F32R,
                         kind="ExternalInput").ap()
    if mode == "masked":
        mask_ext = nc.dram_tensor("mask", [c.S, c.S], F32,
                                  kind="ExternalInput").ap()
    out_ext = nc.dram_tensor("out", [c.TG, c.RS_ROWS, c.hidden], F32,
                             kind="ExternalOutput").ap()

    inv_sqrt_dh = 1.0 / math.sqrt(c.dh)
    NEG = -1.0e9

    with tile.TileContext(nc) as tc:
        with tc.tile_pool(name="dram", bufs=1, space="DRAM") as dram:
            qkvt = dram.tile([c.F, c.T], F32R)
            partials = [dram.tile([c.T // c.TG, c.hidden], F32,
                                  tag=f"part{i}", name=f"part{i}")
                        for i in range(c.TG)]

            # ---------------- Phase A: QKV projection -------------------
            # qkvt[f, t] = sum_h wqkvt[h, f] * xt[h, t]
            do_a, do_b, do_c = ("A" in phases), ("B" in phases), ("C" in phases)
            wq_r = wqkvt.rearrange("(ko p) f -> p ko f", p=128)
            xt_r = xt.rearrange("(ko p) t -> p ko t", p=128)
            qkvt_r = qkvt.rearrange("(ft p) t -> ft p t", p=128)
            splits = _ft_splits(c.F // 128) if do_a else []
            with tc.tile_pool(name="qkv_w", bufs=1) as wpool, \
                 tc.tile_pool(name="qkv_x", bufs=6) as xpool, \
                 tc.tile_pool(name="qkv_o", bufs=8) as opool, \
                 tc.tile_pool(name="qkv_ps", bufs=8, space="PSUM") as pspool:
                ft0 = 0
                for nft in splits:
                    w_sb = wpool.tile([128, c.KT, nft * 128], F32R, tag="w")
                    for kq in range(c.KT):
                        nc.sync.dma_start(
                            w_sb[:, kq],
                            wq_r[:, kq, ft0 * 128:(ft0 + nft) * 128])
                    for tci in range(c.TC):
                        pss = [pspool.tile([128, 512], F32, tag="ps",
                                           name=f"ps{i}")
                               for i in range(nft)]
                        for k in range(c.KT):
                            x_sb = xpool.tile([128, 512], F32R, tag="x")
                            nc.sync.dma_start(
                                x_sb[:], xt_r[:, k, tci * 512:(tci + 1) * 512])
                            for i in range(nft):
                                nc.tensor.matmul(
                                    pss[i][:],
                                    w_sb[:, k, i * 128:(i + 1) * 128],
                                    x_sb[:],
                                    start=(k == 0), stop=(k == c.KT - 1))
                        for i in range(nft):
                            o_sb = opool.tile([128, 512], F32R, tag="o")
                            nc.vector.tensor_copy(o_sb[:], pss[i][:])
                            nc.sync.dma_start(
                                qkvt_r[ft0 + i, :, tci * 512:(tci + 1) * 512],
                                o_sb[:])
                    ft0 += nft

            # ---------------- Phase B: attention ------------------------
            with tc.tile_pool(name="att_at", bufs=1) as atpool:
              attnT = atpool.tile([128, c.FTO, c.T], F32R)
              with tc.tile_pool(name="att_const", bufs=1) as cpool, \
                 tc.tile_pool(name="att_in", bufs=2) as inpool, \
                 tc.tile_pool(name="att_v", bufs=1) as vpool, \
                 tc.tile_pool(name="att_p", bufs=5) as ppool, \
                 tc.tile_pool(name="att_pt", bufs=c.S // 128 + 2) as ptpool, \
                 tc.tile_pool(name="att_sm", bufs=2) as smpool, \
                 tc.tile_pool(name="att_ms", bufs=(4 if mode == "masked" else 1)) as mspool, \
                 tc.tile_pool(name="ps_s", bufs=3, space="PSUM") as ps_s, \
                 tc.tile_pool(name="ps_pt", bufs=2, space="PSUM") as ps_pt, \
                 tc.tile_pool(name="ps_at", bufs=2, space="PSUM") as ps_at, \
                 tc.tile_pool(name="ps_sm", bufs=1, space="PSUM") as ps_sm:

                ident = cpool.tile([128, 128], F32R)
                ones1 = cpool.tile([1, 128], F32R)
                with tc.tile_pool(name="att_tmp", bufs=1) as tmppool:
                    ident32 = tmppool.tile([128, 128], F32)
                    nc.gpsimd.memset(ident32[:], 0.0)
                    nc.gpsimd.affine_select(
                        out=ident32[:], in_=ident32[:],
                        compare_op=mybir.AluOpType.not_equal, fill=1.0,
                        base=0, pattern=[[-1, 128]], channel_multiplier=1)
                    nc.vector.tensor_copy(ident[:], ident32[:])
                    ones32 = tmppool.tile([1, 128], F32, tag="ones32")
                    nc.vector.memset(ones32[:], 1.0)
                    nc.vector.tensor_copy(ones1[:], ones32[:])
                cmasks = []
                if mode == "causal":
                    for off in range(4):
                        m = cpool.tile([128, 512], F32, tag=f"cm{off}",
                                       name=f"cm{off}")
                        nc.gpsimd.memset(m[:], 0.0)
                        # m[p, y] = 0 where y <= off*128 + p else NEG
                        nc.gpsimd.affine_select(
                            out=m[:], in_=m[:],
                            compare_op=mybir.AluOpType.is_ge, fill=NEG,
                            base=off * 128, pattern=[[-1, 512]],
                            channel_multiplier=1)
                        cmasks.append(m)

                for b in range(c.B if do_b else 0):
                    for h in range(c.HL):
                        q_sb = inpool.tile([128, c.S], F32R, tag="q")
                        k_sb = inpool.tile([128, c.S], F32R, tag="k")
                        v_sb = inpool.tile([128, c.S], F32R, tag="v")
                        t0 = b * c.S
                        nc.sync.dma_start(
                            q_sb[:], qkvt[h * 128:(h + 1) * 128, t0:t0 + c.S])
                        nc.sync.dma_start(
                            k_sb[:], qkvt[(c.HL + h) * 128:(c.HL + h + 1) * 128,
                                          t0:t0 + c.S])
                        nc.sync.dma_start(
                            v_sb[:], qkvt[(2 * c.HL + h) * 128:
                                          (2 * c.HL + h + 1) * 128,
                                          t0:t0 + c.S])
                        # V to token-major [128, st, dh]
                        v_tok = vpool.tile([128, c.SQT, 128], F32R)
                        for st in range(c.SQT):
                            vt_ps = ps_sm.tile([128, 128], F32R, tag="sm")
                            nc.tensor.matmul(vt_ps[:],
                                             v_sb[:, st * 128:(st + 1) * 128],
                                             ident[:], is_transpose=True)
                            nc.vector.tensor_copy(v_tok[:, st, :], vt_ps[:])

                        for qc in range(c.QC):
                            if mode == "causal":
                                nkt = 4 * (qc + 1)
                            else:
                                nkt = c.SQT
                            pts = [ptpool.tile([128, 512], F32R, tag="pt",
                                               name=f"pt{i}")
                                   for i in range(nkt)]
                            rqT_ps = ps_sm.tile([1, 512], F32R, tag="sm")
                            for qtl in range(4):
                                qt = qc * 4 + qtl
                                nkc = (qt // 4 + 1) if mode == "causal" \
                                    else c.S // 512
                                dsum = smpool.tile([128, 4], F32, tag="dsum")
                                for kc in range(nkc):
                                    s_ps = ps_s.tile([128, 512], F32, tag="s")
                                    nc.tensor.matmul(
                                        s_ps[:],
                                        q_sb[:, qt * 128:(qt + 1) * 128],
                                        k_sb[:, kc * 512:(kc + 1) * 512],
                                        start=True, stop=True)
                                    if mode == "causal" and kc == nkc - 1:
                                        nc.vector.tensor_tensor(
                                            s_ps[:], s_ps[:],
                                            cmasks[qt % 4][:],
                                            mybir.AluOpType.add)
                                    elif mode == "masked":
                                        m_sb = mspool.tile([128, 512], F32,
                                                           tag="m")
                                        nc.sync.dma_start(
                                            m_sb[:],
                                            mask_ext[qt * 128:(qt + 1) * 128,
                                                     kc * 512:(kc + 1) * 512])
                                        nc.vector.tensor_tensor(
                                            s_ps[:], s_ps[:], m_sb[:],
                                            mybir.AluOpType.add)
                                    p_sb = ppool.tile([128, 512], F32R,
                                                      tag="p")
                                    nc.scalar.activation(
                                        p_sb[:], s_ps[:],
                                        mybir.ActivationFunctionType.Exp,
                                        scale=inv_sqrt_dh,
                                        accum_out=dsum[:, kc:kc + 1])
                                    # transpose the four 128-blocks into pts
                                    for j in range(4):
                                        pt_ps = ps_pt.tile([128, 128], F32R,
                                                           tag="ptp")
                                        nc.tensor.matmul(
                                            pt_ps[:],
                                            p_sb[:, j * 128:(j + 1) * 128],
                                            ident[:], is_transpose=True)
                                        nc.vector.tensor_copy(
                                            pts[kc * 4 + j][:, qtl * 128:
                                                            (qtl + 1) * 128],
                                            pt_ps[:])
                                # 1/rowsum -> transposed into rqT_ps column
                                rqs = smpool.tile([128, 1], F32, tag="rqs")
                                nc.vector.tensor_reduce(
                                    rqs[:], dsum[:, :nkc],
                                    axis=mybir.AxisListType.X,
                                    op=mybir.AluOpType.add)
                                rq = smpool.tile([128, 1], F32, tag="rq")
                                nc.vector.reciprocal(rq[:], rqs[:])
                                rqr = smpool.tile([128, 1], F32R, tag="rqr")
                                nc.vector.tensor_copy(rqr[:], rq[:])
                                nc.tensor.matmul(
                                    rqT_ps[:, qtl * 128:(qtl + 1) * 128],
                                    rqr[:], ident[:], is_transpose=True)
                            rqT_sb = smpool.tile([1, 512], F32R, tag="rqT")
                            nc.vector.tensor_copy(rqT_sb[:], rqT_ps[:])
                            rqb_ps = ps_sm.tile([128, 512], F32, tag="sm")
                            nc.tensor.matmul(rqb_ps[:], ones1[:], rqT_sb[:],
                                             start=True, stop=True)
                            rqb_sb = smpool.tile([128, 512], F32, tag="rqb")
                            nc.vector.tensor_copy(rqb_sb[:], rqb_ps[:])
                            at_ps = ps_at.tile([128, 512], F32, tag="at")
                            for kt in range(nkt):
                                nc.tensor.matmul(
                                    at_ps[:], v_tok[:, kt, :], pts[kt][:],
                                    start=(kt == 0), stop=(kt == nkt - 1))
                            nc.vector.tensor_tensor(
                                attnT[:, h, t0 + qc * 512:t0 + (qc + 1) * 512],
                                at_ps[:], rqb_sb[:], mybir.AluOpType.mult)

              # ---------------- Phase C: o_proj + reduce-scatter ------
              wot_r = wot.rearrange("(ft p) o -> p ft o", p=128)
              with tc.tile_pool(name="op_w", bufs=3) as wopool, \
                   tc.tile_pool(name="op_o", bufs=6) as oopool, \
                   tc.tile_pool(name="op_ps", bufs=4, space="PSUM") as opps:
                  tt_per_g = c.T // c.TG // 128
                  for tg in range(c.TG if do_c else 0):
                      for oc in range(c.OC):
                          wo_sb = wopool.tile([128, c.FTO, 512], F32R,
                                              tag="wo")
                          nc.sync.dma_start(
                              wo_sb[:],
                              wot_r[:, :, oc * 512:(oc + 1) * 512])
                          for tl in range(tt_per_g):
                              tt = tg * tt_per_g + tl
                              ps = opps.tile([128, 512], F32, tag="ops")
                              for ft in range(c.FTO):
                                  nc.tensor.matmul(
                                      ps[:],
                                      attnT[:, ft, tt * 128:(tt + 1) * 128],
                                      wo_sb[:, ft, :],
                                      start=(ft == 0),
                                      stop=(ft == c.FTO - 1))
                              po_sb = oopool.tile([128, 512], F32, tag="po")
                              nc.vector.tensor_copy(po_sb[:], ps[:])
                              nc.sync.dma_start(
                                  partials[tg][tl * 128:(tl + 1) * 128,
                                               oc * 512:(oc + 1) * 512],
                                  po_sb[:])
                      rs_out = dram.tile([c.RS_ROWS, c.hidden], F32,
                                         tag="rs")
                      nc.gpsimd.collective_compute(
                          "ReduceScatter",
                          mybir.AluOpType.add,
                          replica_groups=[list(range(c.n_cores))],
                          ins=[partials[tg][:].opt()],
                          outs=[rs_out[:].opt()],
                      )
                      nc.gpsimd.dma_start(out_ext[tg], rs_out[:])

    nc.compile()
    return nc


# --------------------------------------------------------------------------
_CACHE = {}


def _get_program(cfg: Cfg, mode: str):
    key = (cfg.key(), mode)
    if key not in _CACHE:
        _CACHE[key] = build_program(cfg, mode)
    return _CACHE[key]


def prepare_inputs(cfg: Cfg, hidden_states, attention_mask, W_pack, W_o):
    """Host-side shard + layout prep. Returns (mode, in_maps)."""
    c = cfg
    X = np.asarray(hidden_states, dtype=np.float32).reshape(c.T, c.hidden)
    XT = np.ascontiguousarray(X.T)

    mask = np.asarray(attention_mask, dtype=np.float32).reshape(c.S, c.S)
    causal_ref = np.where(
        np.tril(np.ones((c.S, c.S), dtype=bool)), 0.0, -1e9
    ).astype(np.float32)
    if np.array_equal(mask, causal_ref):
        mode = "causal"
    elif not mask.any():
        mode = "dense"
    else:
        mode = "masked"

    W_pack = np.asarray(W_pack, dtype=np.float32)
    W_o = np.asarray(W_o, dtype=np.float32)
    H = c.hidden
    in_maps = []
    for g in range(c.n_cores):
        r0, r1 = g * c.FO, (g + 1) * c.FO
        wq = W_pack[r0:r1]
        wk = W_pack[H + r0:H + r1]
        wv = W_pack[2 * H + r0:2 * H + r1]
        wqkvT = np.ascontiguousarray(
            np.concatenate([wq, wk, wv], axis=0).T)       # [H, F]
        woT = np.ascontiguousarray(W_o[:, r0:r1].T)       # [FO, H]
        m = {"xt": XT, "wqkvt": wqkvT, "wot": woT}
        if mode == "masked":
            m["mask"] = np.ascontiguousarray(mask * math.sqrt(c.dh))
        in_maps.append(m)
    return mode, in_maps


def assemble_output(cfg: Cfg, results):
    c = cfg
    full = np.empty((c.T, c.hidden), dtype=np.float32)
    rows_g = c.T // c.TG
    for g in range(c.n_cores):
        o = results[g]["out"].reshape(c.TG, c.RS_ROWS, c.hidden)
        for tg in range(c.TG):
            a = tg * rows_g + g * c.RS_ROWS
            full[a:a + c.RS_ROWS] = o[tg]
    return full.reshape(c.B, c.S, c.hidden)


def kernel(hidden_states, attention_mask, W_pack, W_o):
    cfg = Cfg()
    mode, in_maps = prepare_inputs(cfg, hidden_states, attention_mask,
                                   W_pack, W_o)
    nc = _get_program(cfg, mode)
    res = bass_utils.run_bass_kernel_spmd(nc, in_maps,
                                          list(range(cfg.n_cores)))
    return assemble_output(cfg, res.results)



# revision 6
# speedup vs baseline: 1.5462x; 1.5462x over previous
"""Trainium2 Bass kernel for BaichuanAttention (hidden=5120, 40 heads, b=2, s=2048).

Tensor-parallel over heads across 8 NeuronCores, all-bf16 datapath:

  Phase A: per-core QKV projection (5 heads) in bf16, qkv kept resident in
           SBUF (no DRAM round-trip).
  Phase B: flash-style causal attention computing scores TRANSPOSED
           (S^T = K_tile^T Q) so exp() output lands directly in the
           [k, q] layout attn@V needs -- no PE/DVE transposes of P.
           Row-sums via a ones-vector matmul; V transposed to token-major
           with a single XBAR DMA-transpose per head.
  Phase C: AllToAll of the (normalized) per-head attention outputs
           ([T, 640] bf16 per core, 16x less wire than reduce-scattering
           o_proj partials), then each core runs o_proj for its own
           512-token slice against the full W_o.
"""

import math
import sys

for _p in ("/opt/trn_rl_repo",):
    if _p not in sys.path:
        sys.path.insert(0, _p)

import numpy as np
import ml_dtypes

import concourse.bass as bass
import concourse.mybir as mybir
import concourse.tile as tile
from concourse import bacc, bass_utils

F32 = mybir.dt.float32
BF16 = mybir.dt.bfloat16
NPBF16 = ml_dtypes.bfloat16


class Cfg:
    def __init__(self, hidden=5120, n_heads=40, dh=128, B=2, S=2048, n_cores=8):
        self.hidden = hidden
        self.n_heads = n_heads
        self.dh = dh
        self.B = B
        self.S = S
        self.n_cores = n_cores
        assert dh == 128
        self.HL = n_heads // n_cores          # heads per core (5)
        self.F = 3 * self.HL * dh             # per-core packed qkv rows (1920)
        self.FO = self.HL * dh                # per-core attn feature width (640)
        self.T = B * S                        # total tokens (4096)
        self.KT = hidden // 128               # contraction tiles for qkv (40)
        self.FT = self.F // 128               # qkv feature tiles (15)
        self.SQT = S // 128                   # seq 128-tiles per batch (16)
        self.QC = S // 512                    # q chunks per batch (4)
        self.OC = hidden // 512               # o_proj output chunks (10)
        self.BLK = self.T // n_cores          # tokens per core after A2A (512)

    def key(self):
        return (self.hidden, self.n_heads, self.dh, self.B, self.S, self.n_cores)


def build_program(cfg: Cfg, mode: str):
    """mode: 'causal' (mask input ignored, causal skip), 'dense' (zero mask),
    'masked' (general additive mask, host passes mask^T * sqrt(dh))."""
    assert mode in ("causal", "dense", "masked")
    c = cfg
    nc = bacc.Bacc("TRN2", target_bir_lowering=False, debug=False,
                   num_devices=c.n_cores)
    xt = nc.dram_tensor("xt", [c.hidden, c.T], BF16, kind="ExternalInput").ap()
    wqkvt = nc.dram_tensor("wqkvt", [c.hidden, c.F], BF16,
                           kind="ExternalInput").ap()
    wot = nc.dram_tensor("wot", [c.hidden, c.hidden], BF16,
                         kind="ExternalInput").ap()
    maskt = None
    if mode == "masked":
        maskt = nc.dram_tensor("maskt", [c.S, c.S], F32,
                               kind="ExternalInput").ap()
    out_ext = nc.dram_tensor("out", [c.BLK, c.hidden], F32,
                             kind="ExternalOutput").ap()

    inv_sqrt_dh = 1.0 / math.sqrt(c.dh)
    NEG = -1.0e9
    HL, KT, FT, QC, SQT = c.HL, c.KT, c.FT, c.QC, c.SQT

    wq_r = wqkvt.rearrange("(ko p) f -> p ko f", p=128)
    xt_r = xt.rearrange("(ko p) t -> p ko t", p=128)
    wot_r = wot.rearrange("(ko p) o -> p ko o", p=128)
    maskt_r = maskt.rearrange("(kt p) q -> p kt q", p=128) if maskt is not None else None

    with tile.TileContext(nc) as tc:
        with tc.tile_pool(name="dram", bufs=1, space="DRAM") as dram:
            a2a_in = dram.tile([c.n_cores, c.FO, c.BLK], BF16)
            a2a_out = dram.tile([c.n_cores, c.FO, c.BLK], BF16)
            a2a_in_r = a2a_in.rearrange("blk (h p) t -> blk p h t", p=128)
            a2a_out_r = a2a_out.rearrange("g (f p) t -> p (g f) t", p=128)

            with tc.tile_pool(name="const", bufs=1) as cpool:
                ones_col = cpool.tile([128, 1], BF16, tag="onec")
                ones_row = cpool.tile([1, 128], BF16, tag="oner")
                with tc.tile_pool(name="ctmp", bufs=1) as tmp:
                    o32a = tmp.tile([128, 1], F32, tag="o32a")
                    nc.vector.memset(o32a[:], 1.0)
                    nc.vector.tensor_copy(ones_col[:], o32a[:])
                    o32b = tmp.tile([1, 128], F32, tag="o32b")
                    nc.vector.memset(o32b[:], 1.0)
                    nc.vector.tensor_copy(ones_row[:], o32b[:])
                cmasks = []
                if mode == "causal":
                    for off_i in range(4):
                        m = cpool.tile([128, 512], F32, tag=f"cm{off_i}",
                                       name=f"cm{off_i}")
                        nc.gpsimd.memset(m[:], 0.0)
                        # keep 0 where q_local >= k_local: y - p - off >= 0
                        nc.gpsimd.affine_select(
                            out=m[:], in_=m[:],
                            compare_op=mybir.AluOpType.is_ge, fill=NEG,
                            base=-(off_i * 128), pattern=[[1, 512]],
                            channel_multiplier=-1)
                        cmasks.append(m)

                with tc.tile_pool(name="qkvp", bufs=1) as qkvpool, \
                     tc.tile_pool(name="attp", bufs=1) as attpool, \
                     tc.tile_pool(name="vtkp", bufs=2) as vpool:
                    for b in range(c.B):
                        qkv_sb = qkvpool.tile([128, FT, c.S], BF16, tag="qkv",
                                              name=f"qkv{b}")
                        attnT = attpool.tile([128, HL, c.S], BF16, tag="att",
                                             name=f"att{b}")
                        # ---------------- Phase A: QKV projection ---------
                        with tc.tile_pool(name=f"aw{b}", bufs=2) as wpool, \
                             tc.tile_pool(name=f"ax{b}", bufs=2) as xpool, \
                             tc.tile_pool(name=f"aps{b}", bufs=2,
                                          space="PSUM") as apsum:
                            ft0 = 0
                            for nft in [3] * (FT // 3) + ([FT % 3] if FT % 3 else []):
                                w_sb = wpool.tile([128, KT, 3 * 128], BF16,
                                                  tag="w")
                                for ko in range(KT):
                                    nc.scalar.dma_start(
                                        w_sb[:, ko, :nft * 128],
                                        wq_r[:, ko,
                                             ft0 * 128:(ft0 + nft) * 128])
                                for tci in range(QC):
                                    pss = [apsum.tile([128, 512], F32,
                                                      tag=f"aps{i}",
                                                      name=f"aps{i}")
                                           for i in range(nft)]
                                    t0 = b * c.S + tci * 512
                                    for kb in range(KT // 8):
                                        x_sb = xpool.tile([128, 8, 512], BF16,
                                                          tag="x")
                                        nc.sync.dma_start(
                                            x_sb[:],
                                            xt_r[:, kb * 8:(kb + 1) * 8,
                                                 t0:t0 + 512])
                                        for kj in range(8):
                                            ko = kb * 8 + kj
                                            for i in range(nft):
                                                nc.tensor.matmul(
                                                    pss[i][:],
                                                    w_sb[:, ko,
                                                         i * 128:(i + 1) * 128],
                                                    x_sb[:, kj],
                                                    start=(ko == 0),
                                                    stop=(ko == KT - 1))
                                    for i in range(nft):
                                        nc.vector.tensor_copy(
                                            qkv_sb[:, ft0 + i,
                                                   tci * 512:(tci + 1) * 512],
                                            pss[i][:])
                                ft0 += nft

                        # ---------------- Phase B: attention --------------
                        with tc.tile_pool(name=f"bs{b}", bufs=2,
                                          space="PSUM") as spool, \
                             tc.tile_pool(name=f"bat{b}", bufs=2,
                                          space="PSUM") as batp, \
                             tc.tile_pool(name=f"brs{b}", bufs=2,
                                          space="PSUM") as rsbc, \
                             tc.tile_pool(name=f"bp{b}", bufs=9) as ppool, \
                             tc.tile_pool(name=f"bm{b}", bufs=3) as mpool, \
                             tc.tile_pool(name=f"bsm{b}", bufs=2) as smpool:
                            deferred = []

                            def flush():
                                while deferred:
                                    deferred.pop(0)()

                            for h in range(HL):
                                v_tok = vpool.tile([128, SQT, 128], BF16,
                                                   tag="vtok")
                                nc.sync.dma_start_transpose(
                                    v_tok[:], qkv_sb[:, 2 * HL + h, :])
                                for qc in range(QC):
                                    nkp = 2 * (qc + 1) if mode == "causal" \
                                        else SQT // 2
                                    nkt = 2 * nkp
                                    rs_ps = rsbc.tile([1, 512], F32,
                                                      tag="rsbc", name="rs")
                                    at_ps = batp.tile([128, 512], F32,
                                                      tag="at")
                                    pts = []

                                    def emit_one_at(j, rs_ps=rs_ps,
                                                    at_ps=at_ps, pts=pts,
                                                    v_tok=v_tok, nkt=nkt):
                                        for half in range(2):
                                            kt = 2 * j + half
                                            nc.tensor.matmul(
                                                rs_ps[:], ones_col[:],
                                                pts[j][:, half],
                                                start=(kt == 0),
                                                stop=(kt == nkt - 1))
                                            nc.tensor.matmul(
                                                at_ps[:], v_tok[:, kt],
                                                pts[j][:, half],
                                                start=(kt == 0),
                                                stop=(kt == nkt - 1))

                                    for kp in range(nkp):
                                        s_ps = spool.tile([128, 2, 512], F32,
                                                          tag="s")
                                        for half in range(2):
                                            kt = 2 * kp + half
                                            nc.tensor.matmul(
                                                s_ps[:, half],
                                                qkv_sb[:, HL + h,
                                                       kt * 128:(kt + 1) * 128],
                                                qkv_sb[:, h,
                                                       qc * 512:(qc + 1) * 512],
                                                start=True, stop=True)
                                        if mode == "causal" and kp >= 2 * qc:
                                            for half in range(2):
                                                off_i = 2 * kp + half - 4 * qc
                                                nc.vector.tensor_tensor(
                                                    s_ps[:, half],
                                                    s_ps[:, half],
                                                    cmasks[off_i][:],
                                                    mybir.AluOpType.add)
                                        elif mode == "masked":
                                            m_sb = mpool.tile([128, 2, 512],
                                                              F32, tag="m")
                                            nc.sync.dma_start(
                                                m_sb[:],
                                                maskt_r[:, 2 * kp:2 * kp + 2,
                                                        qc * 512:(qc + 1) * 512])
                                            nc.vector.tensor_tensor(
                                                s_ps[:], s_ps[:], m_sb[:],
                                                mybir.AluOpType.add)
                                        p_sb = ppool.tile([128, 2, 512], BF16,
                                                          tag="p")
                                        nc.scalar.activation(
                                            p_sb[:], s_ps[:],
                                            mybir.ActivationFunctionType.Exp,
                                            scale=inv_sqrt_dh)
                                        pts.append(p_sb)
                                        if kp == 1:
                                            flush()
                                        if kp >= 2:
                                            emit_one_at(kp - 2)
                                    if nkp == 1:
                                        flush()
                                    emit_one_at(nkp - 2)
                                    emit_one_at(nkp - 1)

                                    def finalize(h=h, qc=qc, rs_ps=rs_ps,
                                                 at_ps=at_ps, attnT=attnT):
                                        rq32 = smpool.tile([1, 512], F32,
                                                           tag="rq32")
                                        nc.vector.reciprocal(rq32[:], rs_ps[:])
                                        rqbf = smpool.tile([1, 512], BF16,
                                                           tag="rqbf")
                                        nc.vector.tensor_copy(rqbf[:], rq32[:])
                                        bc_ps = rsbc.tile([128, 512], F32,
                                                          tag="rsbc",
                                                          name="bc")
                                        nc.tensor.matmul(bc_ps[:], ones_row[:],
                                                         rqbf[:], start=True,
                                                         stop=True)
                                        rqb_sb = smpool.tile([128, 512], F32,
                                                             tag="rqb")
                                        nc.scalar.copy(rqb_sb[:], bc_ps[:])
                                        nc.vector.tensor_tensor(
                                            attnT[:, h,
                                                  qc * 512:(qc + 1) * 512],
                                            at_ps[:], rqb_sb[:],
                                            mybir.AluOpType.mult)

                                    deferred.append(finalize)
                            flush()
                            # ship this batch's attn to the A2A buffer
                            nblk = c.S // c.BLK
                            for j in range(nblk):
                                nc.sync.dma_start(
                                    a2a_in_r[b * nblk + j],
                                    attnT[:, :, j * c.BLK:(j + 1) * c.BLK])

                # ---------------- Phase C: A2A + o_proj -------------------
                nc.gpsimd.collective_compute(
                    "AllToAll",
                    mybir.AluOpType.bypass,
                    replica_groups=[list(range(c.n_cores))],
                    ins=[a2a_in[:].opt()],
                    outs=[a2a_out[:].opt()],
                )
                with tc.tile_pool(name="catt", bufs=1) as cattp, \
                     tc.tile_pool(name="cwo", bufs=2) as wopool, \
                     tc.tile_pool(name="cout", bufs=4) as outpool, \
                     tc.tile_pool(name="cps", bufs=4, space="PSUM") as cpsum:
                    att_sb = cattp.tile([128, KT, c.BLK], BF16, tag="catt")
                    gstep = max(1, KT // 4)
                    for g0 in range(0, KT, gstep):
                        g1 = min(g0 + gstep, KT)
                        nc.sync.dma_start(
                            att_sb[:, g0:g1],
                            a2a_out_r[:, g0:g1])
                    for oc in range(c.OC):
                        wo_sb = wopool.tile([128, KT, 512], BF16, tag="wo")
                        for ko in range(KT):
                            nc.scalar.dma_start(
                                wo_sb[:, ko],
                                wot_r[:, ko, oc * 512:(oc + 1) * 512])
                        for tt in range(c.BLK // 128):
                            ps = cpsum.tile([128, 512], F32, tag="cps")
                            for ko in range(KT):
                                nc.tensor.matmul(
                                    ps[:],
                                    att_sb[:, ko, tt * 128:(tt + 1) * 128],
                                    wo_sb[:, ko],
                                    start=(ko == 0), stop=(ko == KT - 1))
                            o_sb = outpool.tile([128, 512], F32, tag="o")
                            nc.vector.tensor_copy(o_sb[:], ps[:])
                            nc.sync.dma_start(
                                out_ext[tt * 128:(tt + 1) * 128,
                                        oc * 512:(oc + 1) * 512],
                                o_sb[:])

    nc.compile()
    return nc


# --------------------------------------------------------------------------
_CACHE = {}


def _get_program(cfg: Cfg, mode: str):
    key = (cfg.key(), mode)
    if key not in _CACHE:
        _CACHE[key] = build_program(cfg, mode)
    return _CACHE[key]


def prepare_inputs(cfg: Cfg, hidden_states, attention_mask, W_pack, W_o):
    """Host-side shard + layout prep. Returns (mode, in_maps)."""
    c = cfg
    X = np.asarray(hidden_states, dtype=np.float32).reshape(c.T, c.hidden)
    XT = np.ascontiguousarray(X.T).astype(NPBF16)

    mask = np.asarray(attention_mask, dtype=np.float32).reshape(c.S, c.S)
    causal_ref = np.where(
        np.tril(np.ones((c.S, c.S), dtype=bool)), 0.0, -1e9
    ).astype(np.float32)
    if np.array_equal(mask, causal_ref):
        mode = "causal"
    elif not mask.any():
        mode = "dense"
    else:
        mode = "masked"

    W_pack = np.asarray(W_pack, dtype=np.float32)
    W_o = np.asarray(W_o, dtype=np.float32)
    H = c.hidden
    woT = np.ascontiguousarray(W_o.T).astype(NPBF16)     # [F_in, O]
    maskT = None
    if mode == "masked":
        maskT = np.ascontiguousarray(mask.T * math.sqrt(c.dh),
                                     dtype=np.float32)
    in_maps = []
    for g in range(c.n_cores):
        r0, r1 = g * c.FO, (g + 1) * c.FO
        wq = W_pack[r0:r1]
        wk = W_pack[H + r0:H + r1]
        wv = W_pack[2 * H + r0:2 * H + r1]
        wqkvT = np.ascontiguousarray(
            np.concatenate([wq, wk, wv], axis=0).T).astype(NPBF16)  # [H, F]
        m = {"xt": XT, "wqkvt": wqkvT, "wot": woT}
        if mode == "masked":
            m["maskt"] = maskT
        in_maps.append(m)
    return mode, in_maps


def assemble_output(cfg: Cfg, results):
    c = cfg
    full = np.empty((c.T, c.hidden), dtype=np.float32)
    for g in range(c.n_cores):
        full[g * c.BLK:(g + 1) * c.BLK] = results[g]["out"]
    return full.reshape(c.B, c.S, c.hidden)


def kernel(hidden_states, attention_mask, W_pack, W_o):
    cfg = Cfg()
    mode, in_maps = prepare_inputs(cfg, hidden_states, attention_mask,
                                   W_pack, W_o)
    nc = _get_program(cfg, mode)
    res = bass_utils.run_bass_kernel_spmd(nc, in_maps,
                                          list(range(cfg.n_cores)))
    return assemble_output(cfg, res.results)


# revision 19
# speedup vs baseline: 1.6512x; 1.0679x over previous
"""Trainium2 Bass kernel for BaichuanAttention (hidden=5120, 40 heads, b=2, s=2048).

Tensor-parallel over heads across 8 NeuronCores, all-bf16 datapath:

  Phase A: per-core QKV projection (5 heads) in bf16, qkv kept resident in
           SBUF (no DRAM round-trip).
  Phase B: flash-style causal attention computing scores TRANSPOSED
           (S^T = K_tile^T Q) so exp() output lands directly in the
           [k, q] layout attn@V needs -- no PE/DVE transposes of P.
           Row-sums via a ones-vector matmul; V transposed to token-major
           with a single XBAR DMA-transpose per head.
  Phase C: AllToAll of the (normalized) per-head attention outputs
           ([T, 640] bf16 per core, 16x less wire than reduce-scattering
           o_proj partials), then each core runs o_proj for its own
           512-token slice against the full W_o.
"""

import math
import sys

for _p in ("/opt/trn_rl_repo",):
    if _p not in sys.path:
        sys.path.insert(0, _p)

import numpy as np
import ml_dtypes

import concourse.bass as bass
import concourse.mybir as mybir
import concourse.tile as tile
from concourse import bacc, bass_utils

F32 = mybir.dt.float32
BF16 = mybir.dt.bfloat16
NPBF16 = ml_dtypes.bfloat16


class Cfg:
    def __init__(self, hidden=5120, n_heads=40, dh=128, B=2, S=2048, n_cores=8):
        self.hidden = hidden
        self.n_heads = n_heads
        self.dh = dh
        self.B = B
        self.S = S
        self.n_cores = n_cores
        assert dh == 128
        self.HL = n_heads // n_cores          # heads per core (5)
        self.F = 3 * self.HL * dh             # per-core packed qkv rows (1920)
        self.FO = self.HL * dh                # per-core attn feature width (640)
        self.T = B * S                        # total tokens (4096)
        self.KT = hidden // 128               # contraction tiles for qkv (40)
        self.FT = self.F // 128               # qkv feature tiles (15)
        self.SQT = S // 128                   # seq 128-tiles per batch (16)
        self.QC = S // 512                    # q chunks per batch (4)
        self.OC = hidden // 512               # o_proj output chunks (10)
        self.BLK = self.T // n_cores          # tokens per core after A2A (512)

    def key(self):
        return (self.hidden, self.n_heads, self.dh, self.B, self.S, self.n_cores)


def build_program(cfg: Cfg, mode: str):
    """mode: 'causal' (mask input ignored, causal skip), 'dense' (zero mask),
    'masked' (general additive mask, host passes mask^T * sqrt(dh))."""
    assert mode in ("causal", "dense", "masked")
    c = cfg
    nc = bacc.Bacc("TRN2", target_bir_lowering=False, debug=False,
                   num_devices=c.n_cores)
    NG = c.FT // 3                            # phase A ft-groups of 3
    assert c.FT % 3 == 0
    xt = nc.dram_tensor("xt", [c.hidden, c.T], BF16, kind="ExternalInput").ap()
    # W_pack shard, pre-grouped on host: [128, NG, KT, 384] so each group is
    # one fully-contiguous-per-partition DMA
    wg = nc.dram_tensor("wg", [128, NG, c.KT, 3 * 128], BF16,
                        kind="ExternalInput").ap()
    # full W_o^T, pre-grouped: [128, OC, KT, 512]
    wog = nc.dram_tensor("wog", [128, c.OC, c.KT, 512], BF16,
                         kind="ExternalInput").ap()
    maskt = None
    if mode == "masked":
        maskt = nc.dram_tensor("maskt", [c.S, c.S], F32,
                               kind="ExternalInput").ap()
    out_ext = nc.dram_tensor("out", [c.BLK, c.hidden], F32,
                             kind="ExternalOutput").ap()

    inv_sqrt_dh = 1.0 / math.sqrt(c.dh)
    NEG = -1.0e9
    HL, KT, FT, QC, SQT = c.HL, c.KT, c.FT, c.QC, c.SQT

    xt_r = xt.rearrange("(ko p) t -> p ko t", p=128)
    maskt_r = maskt.rearrange("(kt p) q -> p kt q", p=128) if maskt is not None else None
    CH = min(1024, c.S)                       # phase A token chunk
    NH2 = CH // 512

    # per-batch A2A: each batch's attn output is exchanged separately so the
    # collective overlaps with the other batch's compute. Rank i owns token
    # rows [i*HB, (i+1)*HB) of each batch (HB = S / n_cores).
    HB = c.S // c.n_cores
    with tile.TileContext(nc) as tc:
        with tc.tile_pool(name="dram", bufs=1, space="DRAM") as dram:
            a2a_ins = [dram.tile([c.n_cores, c.FO, HB], BF16,
                                 tag=f"a2ai{b}", name=f"a2ai{b}")
                       for b in range(c.B)]
            a2a_outs = [dram.tile([c.n_cores, c.FO, HB], BF16,
                                  tag=f"a2ao{b}", name=f"a2ao{b}")
                        for b in range(c.B)]
            a2a_in_rs = [t.rearrange("blk (h p) t -> blk p h t", p=128)
                         for t in a2a_ins]
            a2a_out_rs = [t.rearrange("g (f p) t -> p (g f) t", p=128)
                          for t in a2a_outs]

            with tc.tile_pool(name="const", bufs=1) as cpool:
                ones_col = cpool.tile([128, 1], BF16, tag="onec")
                ones_row = cpool.tile([1, 128], BF16, tag="oner")
                with tc.tile_pool(name="ctmp", bufs=1) as tmp:
                    o32a = tmp.tile([128, 1], F32, tag="o32a")
                    nc.vector.memset(o32a[:], 1.0)
                    nc.vector.tensor_copy(ones_col[:], o32a[:])
                    o32b = tmp.tile([1, 128], F32, tag="o32b")
                    nc.vector.memset(o32b[:], 1.0)
                    nc.vector.tensor_copy(ones_row[:], o32b[:])
                cmask = None
                if mode == "causal":
                    # triangular 128x128 mask: 0 where q >= k, else NEG
                    cmask = cpool.tile([128, 128], F32, tag="cmask")
                    nc.gpsimd.memset(cmask[:], 0.0)
                    nc.gpsimd.affine_select(
                        out=cmask[:], in_=cmask[:],
                        compare_op=mybir.AluOpType.is_ge, fill=NEG,
                        base=0, pattern=[[1, 128]],
                        channel_multiplier=-1)

                with tc.tile_pool(name="qkvp", bufs=1) as qkvpool, \
                     tc.tile_pool(name="attp", bufs=1) as attpool, \
                     tc.tile_pool(name="vtkp", bufs=2) as vpool:
                    for b in range(c.B):
                        qkv_sb = qkvpool.tile([128, FT, c.S], BF16, tag="qkv",
                                              name=f"qkv{b}")
                        attnT = attpool.tile([128, HL, c.S], BF16, tag="att",
                                             name=f"att{b}")
                        # ---------------- Phase A: QKV projection ---------
                        with tc.tile_pool(name=f"aw{b}", bufs=2) as wpool, \
                             tc.tile_pool(name=f"ax{b}", bufs=2) as xpool, \
                             tc.tile_pool(name=f"aps{b}", bufs=1,
                                          space="PSUM") as apsum:
                            for g in range(NG):
                                ft0 = g * 3
                                w_sb = wpool.tile([128, KT, 3 * 128], BF16,
                                                  tag="w")
                                nc.scalar.dma_start(w_sb[:], wg[:, g])
                                for ci in range(c.S // CH):
                                    pss = [apsum.tile([128, 512], F32,
                                                      tag=f"aps{u}",
                                                      name=f"aps{u}")
                                           for u in range(3 * NH2)]
                                    t0 = b * c.S + ci * CH
                                    for kb in range(KT // 8):
                                        x_sb = xpool.tile([128, 8, CH], BF16,
                                                          tag="x")
                                        eng = nc.sync if kb % 2 == 0 \
                                            else nc.scalar
                                        eng.dma_start(
                                            x_sb[:],
                                            xt_r[:, kb * 8:(kb + 1) * 8,
                                                 t0:t0 + CH])
                                        for kj in range(8):
                                            ko = kb * 8 + kj
                                            for i in range(3):
                                                for hf in range(NH2):
                                                    nc.tensor.matmul(
                                                        pss[i * NH2 + hf][:],
                                                        w_sb[:, ko,
                                                             i * 128:
                                                             (i + 1) * 128],
                                                        x_sb[:, kj,
                                                             hf * 512:
                                                             (hf + 1) * 512],
                                                        start=(ko == 0),
                                                        stop=(ko == KT - 1))
                                    for i in range(3):
                                        for hf in range(NH2):
                                            o0 = ci * CH + hf * 512
                                            nc.vector.tensor_copy(
                                                qkv_sb[:, ft0 + i,
                                                       o0:o0 + 512],
                                                pss[i * NH2 + hf][:])

                        # ---------------- Phase B: attention --------------
                        with tc.tile_pool(name=f"bs{b}", bufs=2,
                                          space="PSUM") as spool, \
                             tc.tile_pool(name=f"bat{b}", bufs=2,
                                          space="PSUM") as batp, \
                             tc.tile_pool(name=f"brs{b}", bufs=2,
                                          space="PSUM") as rsbc, \
                             tc.tile_pool(name=f"bp{b}", bufs=9) as ppool, \
                             tc.tile_pool(name=f"bm{b}", bufs=3) as mpool, \
                             tc.tile_pool(name=f"bsm{b}", bufs=2) as smpool:
                            deferred = []

                            def flush():
                                while deferred:
                                    deferred.pop(0)()

                            for h in range(HL):
                                v_tok = vpool.tile([128, SQT, 128], BF16,
                                                   tag="vtok")
                                nc.sync.dma_start_transpose(
                                    v_tok[:], qkv_sb[:, 2 * HL + h, :])
                                for qc in range(QC):
                                    nkp = 2 * (qc + 1) if mode == "causal" \
                                        else SQT // 2
                                    nkt = 2 * nkp
                                    rs_ps = rsbc.tile([1, 512], F32,
                                                      tag="rsbc", name="rs")
                                    at_ps = batp.tile([128, 512], F32,
                                                      tag="at")
                                    pts = []

                                    def lo_of(kt):
                                        # first valid q column of k-tile kt
                                        if mode != "causal":
                                            return 0
                                        return max(0, kt * 128 - qc * 512)

                                    def emit_one_at(j, rs_ps=rs_ps,
                                                    at_ps=at_ps, pts=pts,
                                                    v_tok=v_tok, nkt=nkt,
                                                    lo_of=lo_of):
                                        for half in range(2):
                                            kt = 2 * j + half
                                            lo = lo_of(kt)
                                            nc.tensor.matmul(
                                                rs_ps[:, lo:], ones_col[:],
                                                pts[j][:, half, lo:],
                                                start=(kt == 0),
                                                stop=(kt == nkt - 1))
                                            nc.tensor.matmul(
                                                at_ps[:, lo:], v_tok[:, kt],
                                                pts[j][:, half, lo:],
                                                start=(kt == 0),
                                                stop=(kt == nkt - 1))

                                    for kp in range(nkp):
                                        s_ps = spool.tile([128, 2, 512], F32,
                                                          tag="s")
                                        for half in range(2):
                                            kt = 2 * kp + half
                                            lo = lo_of(kt)
                                            nc.tensor.matmul(
                                                s_ps[:, half],
                                                qkv_sb[:, HL + h,
                                                       kt * 128:(kt + 1) * 128],
                                                qkv_sb[:, h,
                                                       qc * 512:(qc + 1) * 512],
                                                start=True, stop=True)
                                            if mode == "causal" \
                                                    and kt >= 4 * qc:
                                                # diagonal: triangular mask on
                                                # q columns [lo, lo+128)
                                                nc.vector.tensor_tensor(
                                                    s_ps[:, half,
                                                         lo:lo + 128],
                                                    s_ps[:, half,
                                                         lo:lo + 128],
                                                    cmask[:],
                                                    mybir.AluOpType.add)
                                        if mode == "masked":
                                            m_sb = mpool.tile([128, 2, 512],
                                                              F32, tag="m")
                                            nc.sync.dma_start(
                                                m_sb[:],
                                                maskt_r[:, 2 * kp:2 * kp + 2,
                                                        qc * 512:(qc + 1) * 512])
                                            nc.vector.tensor_tensor(
                                                s_ps[:], s_ps[:], m_sb[:],
                                                mybir.AluOpType.add)
                                        p_sb = ppool.tile([128, 2, 512], BF16,
                                                          tag="p")
                                        nc.scalar.activation(
                                            p_sb[:], s_ps[:],
                                            mybir.ActivationFunctionType.Exp,
                                            scale=inv_sqrt_dh)
                                        pts.append(p_sb)
                                        if kp == 1:
                                            flush()
                                        if kp >= 2:
                                            emit_one_at(kp - 2)
                                    if nkp == 1:
                                        flush()
                                    emit_one_at(nkp - 2)
                                    emit_one_at(nkp - 1)

                                    def finalize(h=h, qc=qc, rs_ps=rs_ps,
                                                 at_ps=at_ps, attnT=attnT):
                                        # broadcast the row-sums to all 128
                                        # partitions first, then reciprocal
                                        # runs on 128 lanes instead of 1
                                        rsbf = smpool.tile([1, 512], BF16,
                                                           tag="rsbf")
                                        nc.scalar.copy(rsbf[:], rs_ps[:])
                                        bc_ps = rsbc.tile([128, 512], F32,
                                                          tag="rsbc",
                                                          name="bc")
                                        nc.tensor.matmul(bc_ps[:], ones_row[:],
                                                         rsbf[:], start=True,
                                                         stop=True)
                                        rqb_sb = smpool.tile([128, 512], F32,
                                                             tag="rqb")
                                        nc.vector.reciprocal(rqb_sb[:],
                                                             bc_ps[:])
                                        nc.vector.tensor_tensor(
                                            attnT[:, h,
                                                  qc * 512:(qc + 1) * 512],
                                            at_ps[:], rqb_sb[:],
                                            mybir.AluOpType.mult)

                                    deferred.append(finalize)
                            flush()
                            # ship this batch's attn to its A2A buffer
                            for j in range(c.n_cores):
                                nc.sync.dma_start(
                                    a2a_in_rs[b][j],
                                    attnT[:, :, j * HB:(j + 1) * HB])
                        # per-batch AllToAll; b0's overlaps with b1 compute
                        nc.gpsimd.collective_compute(
                            "AllToAll",
                            mybir.AluOpType.bypass,
                            replica_groups=[list(range(c.n_cores))],
                            ins=[a2a_ins[b][:].opt()],
                            outs=[a2a_outs[b][:].opt()],
                        )

                # ---------------- Phase C: o_proj -------------------------
                with tc.tile_pool(name="catt", bufs=1) as cattp, \
                     tc.tile_pool(name="cwo", bufs=2) as wopool, \
                     tc.tile_pool(name="cout", bufs=4) as outpool, \
                     tc.tile_pool(name="cps", bufs=4, space="PSUM") as cpsum:
                    # my tokens: [0:HB) from batch 0, [HB:2*HB) from batch 1
                    att_sb = cattp.tile([128, KT, c.BLK], BF16, tag="catt")
                    gstep = max(1, KT // 4)
                    for bb in range(c.B):
                        for g0 in range(0, KT, gstep):
                            g1 = min(g0 + gstep, KT)
                            nc.sync.dma_start(
                                att_sb[:, g0:g1, bb * HB:(bb + 1) * HB],
                                a2a_out_rs[bb][:, g0:g1])
                    for oc in range(c.OC):
                        wo_sb = wopool.tile([128, KT, 512], BF16, tag="wo")
                        nc.scalar.dma_start(wo_sb[:], wog[:, oc])
                        for tt in range(c.BLK // 128):
                            ps = cpsum.tile([128, 512], F32, tag="cps")
                            for ko in range(KT):
                                nc.tensor.matmul(
                                    ps[:],
                                    att_sb[:, ko, tt * 128:(tt + 1) * 128],
                                    wo_sb[:, ko],
                                    start=(ko == 0), stop=(ko == KT - 1))
                            o_sb = outpool.tile([128, 512], F32, tag="o")
                            nc.vector.tensor_copy(o_sb[:], ps[:])
                            nc.sync.dma_start(
                                out_ext[tt * 128:(tt + 1) * 128,
                                        oc * 512:(oc + 1) * 512],
                                o_sb[:])

    nc.compile()
    return nc


# --------------------------------------------------------------------------
_CACHE = {}


def _get_program(cfg: Cfg, mode: str):
    key = (cfg.key(), mode)
    if key not in _CACHE:
        _CACHE[key] = build_program(cfg, mode)
    return _CACHE[key]


def prepare_inputs(cfg: Cfg, hidden_states, attention_mask, W_pack, W_o):
    """Host-side shard + layout prep. Returns (mode, in_maps)."""
    c = cfg
    X = np.asarray(hidden_states, dtype=np.float32).reshape(c.T, c.hidden)
    XT = np.ascontiguousarray(X.T).astype(NPBF16)

    mask = np.asarray(attention_mask, dtype=np.float32).reshape(c.S, c.S)
    causal_ref = np.where(
        np.tril(np.ones((c.S, c.S), dtype=bool)), 0.0, -1e9
    ).astype(np.float32)
    if np.array_equal(mask, causal_ref):
        mode = "causal"
    elif not mask.any():
        mode = "dense"
    else:
        mode = "masked"

    W_pack = np.asarray(W_pack, dtype=np.float32)
    W_o = np.asarray(W_o, dtype=np.float32)
    H, KT, OC = c.hidden, c.KT, c.OC
    # full W_o^T grouped for phase C: [128, OC, KT, 512]
    wog = np.ascontiguousarray(
        W_o.T.reshape(KT, 128, OC, 512).transpose(1, 2, 0, 3)).astype(NPBF16)
    maskT = None
    if mode == "masked":
        maskT = np.ascontiguousarray(mask.T * math.sqrt(c.dh),
                                     dtype=np.float32)
    NG = c.FT // 3
    in_maps = []
    for g in range(c.n_cores):
        r0, r1 = g * c.FO, (g + 1) * c.FO
        wq = W_pack[r0:r1]
        wk = W_pack[H + r0:H + r1]
        wv = W_pack[2 * H + r0:2 * H + r1]
        wqkvT = np.concatenate([wq, wk, wv], axis=0).T   # [H, F]
        # grouped for phase A: [128, NG, KT, 384]
        wgg = np.ascontiguousarray(
            wqkvT.reshape(KT, 128, NG, 384).transpose(1, 2, 0, 3)
        ).astype(NPBF16)
        m = {"xt": XT, "wg": wgg, "wog": wog}
        if mode == "masked":
            m["maskt"] = maskT
        in_maps.append(m)
    return mode, in_maps


def assemble_output(cfg: Cfg, results):
    c = cfg
    HB = c.S // c.n_cores
    full = np.empty((c.T, c.hidden), dtype=np.float32)
    for g in range(c.n_cores):
        o = results[g]["out"]
        for b in range(c.B):
            full[b * c.S + g * HB:b * c.S + (g + 1) * HB] = \
                o[b * HB:(b + 1) * HB]
    return full.reshape(c.B, c.S, c.hidden)


def kernel(hidden_states, attention_mask, W_pack, W_o):
    cfg = Cfg()
    mode, in_maps = prepare_inputs(cfg, hidden_states, attention_mask,
                                   W_pack, W_o)
    nc = _get_program(cfg, mode)
    res = bass_utils.run_bass_kernel_spmd(nc, in_maps,
                                          list(range(cfg.n_cores)))
    return assemble_output(cfg, res.results)


# revision 30
# speedup vs baseline: 1.6722x; 1.0127x over previous
"""Trainium2 Bass kernel for BaichuanAttention (hidden=5120, 40 heads, b=2, s=2048).

Tensor-parallel over heads across 8 NeuronCores, all-bf16 datapath:

  Phase A: per-core QKV projection (5 heads) in bf16, qkv kept resident in
           SBUF (no DRAM round-trip).
  Phase B: flash-style causal attention computing scores TRANSPOSED
           (S^T = K_tile^T Q) so exp() output lands directly in the
           [k, q] layout attn@V needs -- no PE/DVE transposes of P.
           Row-sums via a ones-vector matmul; V transposed to token-major
           with a single XBAR DMA-transpose per head.
  Phase C: AllToAll of the (normalized) per-head attention outputs
           ([T, 640] bf16 per core, 16x less wire than reduce-scattering
           o_proj partials), then each core runs o_proj for its own
           512-token slice against the full W_o.
"""

import math
import sys

for _p in ("/opt/trn_rl_repo",):
    if _p not in sys.path:
        sys.path.insert(0, _p)

import numpy as np
import ml_dtypes

import concourse.bass as bass
import concourse.mybir as mybir
import concourse.tile as tile
from concourse import bacc, bass_utils

F32 = mybir.dt.float32
BF16 = mybir.dt.bfloat16
NPBF16 = ml_dtypes.bfloat16


class Cfg:
    def __init__(self, hidden=5120, n_heads=40, dh=128, B=2, S=2048, n_cores=8):
        self.hidden = hidden
        self.n_heads = n_heads
        self.dh = dh
        self.B = B
        self.S = S
        self.n_cores = n_cores
        assert dh == 128
        self.HL = n_heads // n_cores          # heads per core (5)
        self.F = 3 * self.HL * dh             # per-core packed qkv rows (1920)
        self.FO = self.HL * dh                # per-core attn feature width (640)
        self.T = B * S                        # total tokens (4096)
        self.KT = hidden // 128               # contraction tiles for qkv (40)
        self.FT = self.F // 128               # qkv feature tiles (15)
        self.SQT = S // 128                   # seq 128-tiles per batch (16)
        self.QC = S // 512                    # q chunks per batch (4)
        self.OC = hidden // 512               # o_proj output chunks (10)
        self.BLK = self.T // n_cores          # tokens per core after A2A (512)

    def key(self):
        return (self.hidden, self.n_heads, self.dh, self.B, self.S, self.n_cores)


def build_program(cfg: Cfg, mode: str):
    """mode: 'causal' (mask input ignored, causal skip), 'dense' (zero mask),
    'masked' (general additive mask, host passes mask^T * sqrt(dh))."""
    assert mode in ("causal", "dense", "masked")
    c = cfg
    nc = bacc.Bacc("TRN2", target_bir_lowering=False, debug=False,
                   num_devices=c.n_cores)
    NG = c.FT // 3                            # phase A ft-groups of 3
    assert c.FT % 3 == 0
    CH0 = min(1024, c.S)
    # X^T chunked on host: [128, T/CH, KT, CH] -> 16KB-contiguous DMA lines
    xg_t = nc.dram_tensor("xg", [128, c.T // CH0, c.KT, CH0], BF16,
                          kind="ExternalInput").ap()
    # W_pack shard, pre-grouped on host: [128, NG, KT, 384] so each group is
    # one fully-contiguous-per-partition DMA
    wg = nc.dram_tensor("wg", [128, NG, c.KT, 3 * 128], BF16,
                        kind="ExternalInput").ap()
    # full W_o^T, pre-grouped: [128, OC, KT, 512]
    wog = nc.dram_tensor("wog", [128, c.OC, c.KT, 512], BF16,
                         kind="ExternalInput").ap()
    maskt = None
    if mode == "masked":
        maskt = nc.dram_tensor("maskt", [c.S, c.S], F32,
                               kind="ExternalInput").ap()
    out_ext = nc.dram_tensor("out", [c.BLK, c.hidden], F32,
                             kind="ExternalOutput").ap()

    inv_sqrt_dh = 1.0 / math.sqrt(c.dh)
    NEG = -1.0e9
    HL, KT, FT, QC, SQT = c.HL, c.KT, c.FT, c.QC, c.SQT

    maskt_r = maskt.rearrange("(kt p) q -> p kt q", p=128) if maskt is not None else None
    CH = CH0                                  # phase A token chunk
    NH2 = CH // 512

    # per-batch A2A: each batch's attn output is exchanged separately so the
    # collective overlaps with the other batch's compute. Rank i owns token
    # rows [i*HB, (i+1)*HB) of each batch (HB = S / n_cores).
    HB = c.S // c.n_cores
    with tile.TileContext(nc) as tc:
        with tc.tile_pool(name="dram", bufs=1, space="DRAM") as dram:
            a2a_ins = [dram.tile([c.n_cores, c.FO, HB], BF16,
                                 tag=f"a2ai{b}", name=f"a2ai{b}")
                       for b in range(c.B)]
            a2a_outs = [dram.tile([c.n_cores, c.FO, HB], BF16,
                                  tag=f"a2ao{b}", name=f"a2ao{b}")
                        for b in range(c.B)]
            a2a_in_rs = [t.rearrange("blk (h p) t -> blk p h t", p=128)
                         for t in a2a_ins]
            a2a_out_rs = [t.rearrange("g (f p) t -> p (g f) t", p=128)
                          for t in a2a_outs]

            with tc.tile_pool(name="const", bufs=1) as cpool:
                ones_col = cpool.tile([128, 1], BF16, tag="onec")
                ones_row = cpool.tile([1, 128], BF16, tag="oner")
                with tc.tile_pool(name="ctmp", bufs=1) as tmp:
                    o32a = tmp.tile([128, 1], F32, tag="o32a")
                    nc.vector.memset(o32a[:], 1.0)
                    nc.vector.tensor_copy(ones_col[:], o32a[:])
                    o32b = tmp.tile([1, 128], F32, tag="o32b")
                    nc.vector.memset(o32b[:], 1.0)
                    nc.vector.tensor_copy(ones_row[:], o32b[:])
                cmask = None
                if mode == "causal":
                    # triangular 128x128 mask: 0 where q >= k, else NEG
                    cmask = cpool.tile([128, 128], F32, tag="cmask")
                    nc.gpsimd.memset(cmask[:], 0.0)
                    nc.gpsimd.affine_select(
                        out=cmask[:], in_=cmask[:],
                        compare_op=mybir.AluOpType.is_ge, fill=NEG,
                        base=0, pattern=[[1, 128]],
                        channel_multiplier=-1)

                with tc.tile_pool(name="qkvp", bufs=1) as qkvpool, \
                     tc.tile_pool(name="attp", bufs=1) as attpool, \
                     tc.tile_pool(name="vtkp", bufs=2) as vpool, \
                     tc.tile_pool(name="aw", bufs=2) as wpool, \
                     tc.tile_pool(name="ax", bufs=2) as xpool:
                    pending_w = {}
                    pending_x = {}

                    def load_w(g):
                        w_sb = wpool.tile([128, KT, 3 * 128], BF16, tag="w")
                        hk = KT // 2
                        nc.sync.dma_start(w_sb[:, :hk], wg[:, g, :hk])
                        nc.scalar.dma_start(w_sb[:, hk:], wg[:, g, hk:])
                        return w_sb

                    def load_x1(ci_g, kb):
                        # one [128, 8, CH] chunk, split across both DMA queues
                        x_sb = xpool.tile([128, 8, CH], BF16, tag="x")
                        nc.sync.dma_start(
                            x_sb[:, :4], xg_t[:, ci_g, kb * 8:kb * 8 + 4])
                        nc.scalar.dma_start(
                            x_sb[:, 4:], xg_t[:, ci_g, kb * 8 + 4:kb * 8 + 8])
                        return x_sb

                    for b in range(c.B):
                        qkv_sb = qkvpool.tile([128, FT, c.S], BF16, tag="qkv",
                                              name=f"qkv{b}")
                        attnT = attpool.tile([128, HL, c.S], BF16, tag="att",
                                             name=f"att{b}")
                        # ---------------- Phase A: QKV projection ---------
                        with tc.tile_pool(name=f"aps{b}", bufs=1,
                                          space="PSUM") as apsum:
                            for g in range(NG):
                                ft0 = g * 3
                                w_sb = pending_w.pop(b, None) if g == 0 \
                                    else None
                                if w_sb is None:
                                    w_sb = load_w(g)
                                for ci in range(c.S // CH):
                                    ci_g = (b * c.S + ci * CH) // CH
                                    pss = [apsum.tile([128, 512], F32,
                                                      tag=f"aps{u}",
                                                      name=f"aps{u}")
                                           for u in range(3 * NH2)]
                                    for kb in range(KT // 8):
                                        x_sb = None
                                        if g == 0 and ci == 0 and kb == 0:
                                            x_sb = pending_x.pop(b, None)
                                        if x_sb is None:
                                            x_sb = load_x1(ci_g, kb)
                                        for kj in range(8):
                                            ko = kb * 8 + kj
                                            for i in range(3):
                                                for hf in range(NH2):
                                                    nc.tensor.matmul(
                                                        pss[i * NH2 + hf][:],
                                                        w_sb[:, ko,
                                                             i * 128:
                                                             (i + 1) * 128],
                                                        x_sb[:, kj,
                                                             hf * 512:
                                                             (hf + 1) * 512],
                                                        start=(ko == 0),
                                                        stop=(ko == KT - 1))
                                    for i in range(3):
                                        for hf in range(NH2):
                                            o0 = ci * CH + hf * 512
                                            nc.vector.tensor_copy(
                                                qkv_sb[:, ft0 + i,
                                                       o0:o0 + 512],
                                                pss[i * NH2 + hf][:])

                        # ---------------- Phase B: attention --------------
                        with tc.tile_pool(name=f"bs{b}", bufs=2,
                                          space="PSUM") as spool, \
                             tc.tile_pool(name=f"bat{b}", bufs=2,
                                          space="PSUM") as batp, \
                             tc.tile_pool(name=f"brs{b}", bufs=2,
                                          space="PSUM") as rsbc, \
                             tc.tile_pool(name=f"bp{b}", bufs=9) as ppool, \
                             tc.tile_pool(name=f"bm{b}", bufs=3) as mpool, \
                             tc.tile_pool(name=f"bsm{b}", bufs=2) as smpool:
                            deferred = []

                            def flush():
                                while deferred:
                                    deferred.pop(0)()

                            for h in range(HL):
                                v_tok = vpool.tile([128, SQT, 128], BF16,
                                                   tag="vtok")
                                nc.sync.dma_start_transpose(
                                    v_tok[:], qkv_sb[:, 2 * HL + h, :])
                                for qc in range(QC):
                                    nkp = 2 * (qc + 1) if mode == "causal" \
                                        else SQT // 2
                                    nkt = 2 * nkp
                                    rs_ps = rsbc.tile([1, 512], F32,
                                                      tag="rsbc", name="rs")
                                    at_ps = batp.tile([128, 512], F32,
                                                      tag="at")
                                    pts = []

                                    def lo_of(kt):
                                        # first valid q column of k-tile kt
                                        if mode != "causal":
                                            return 0
                                        return max(0, kt * 128 - qc * 512)

                                    def emit_one_at(j, rs_ps=rs_ps,
                                                    at_ps=at_ps, pts=pts,
                                                    v_tok=v_tok, nkt=nkt,
                                                    lo_of=lo_of):
                                        for half in range(2):
                                            kt = 2 * j + half
                                            lo = lo_of(kt)
                                            nc.tensor.matmul(
                                                rs_ps[:, lo:], ones_col[:],
                                                pts[j][:, half, lo:],
                                                start=(kt == 0),
                                                stop=(kt == nkt - 1))
                                            nc.tensor.matmul(
                                                at_ps[:, lo:], v_tok[:, kt],
                                                pts[j][:, half, lo:],
                                                start=(kt == 0),
                                                stop=(kt == nkt - 1))

                                    for kp in range(nkp):
                                        s_ps = spool.tile([128, 2, 512], F32,
                                                          tag="s")
                                        for half in range(2):
                                            kt = 2 * kp + half
                                            lo = lo_of(kt)
                                            nc.tensor.matmul(
                                                s_ps[:, half],
                                                qkv_sb[:, HL + h,
                                                       kt * 128:(kt + 1) * 128],
                                                qkv_sb[:, h,
                                                       qc * 512:(qc + 1) * 512],
                                                start=True, stop=True)
                                            if mode == "causal" \
                                                    and kt >= 4 * qc:
                                                # diagonal: triangular mask on
                                                # q columns [lo, lo+128)
                                                nc.vector.tensor_tensor(
                                                    s_ps[:, half,
                                                         lo:lo + 128],
                                                    s_ps[:, half,
                                                         lo:lo + 128],
                                                    cmask[:],
                                                    mybir.AluOpType.add)
                                        if mode == "masked":
                                            m_sb = mpool.tile([128, 2, 512],
                                                              F32, tag="m")
                                            nc.sync.dma_start(
                                                m_sb[:],
                                                maskt_r[:, 2 * kp:2 * kp + 2,
                                                        qc * 512:(qc + 1) * 512])
                                            nc.vector.tensor_tensor(
                                                s_ps[:], s_ps[:], m_sb[:],
                                                mybir.AluOpType.add)
                                        p_sb = ppool.tile([128, 2, 512], BF16,
                                                          tag="p")
                                        nc.scalar.activation(
                                            p_sb[:], s_ps[:],
                                            mybir.ActivationFunctionType.Exp,
                                            scale=inv_sqrt_dh)
                                        pts.append(p_sb)
                                        if kp == 1:
                                            flush()
                                        if kp >= 2:
                                            emit_one_at(kp - 2)
                                    if nkp == 1:
                                        flush()
                                    emit_one_at(nkp - 2)
                                    emit_one_at(nkp - 1)

                                    def finalize(h=h, qc=qc, rs_ps=rs_ps,
                                                 at_ps=at_ps, attnT=attnT,
                                                 b=b):
                                        # broadcast the row-sums to all 128
                                        # partitions first, then reciprocal
                                        # runs on 128 lanes instead of 1
                                        rsbf = smpool.tile([1, 512], BF16,
                                                           tag="rsbf")
                                        nc.scalar.copy(rsbf[:], rs_ps[:])
                                        bc_ps = rsbc.tile([128, 512], F32,
                                                          tag="rsbc",
                                                          name="bc")
                                        nc.tensor.matmul(bc_ps[:], ones_row[:],
                                                         rsbf[:], start=True,
                                                         stop=True)
                                        rqb_sb = smpool.tile([128, 512], F32,
                                                             tag="rqb")
                                        nc.vector.reciprocal_approx_fast(
                                            rqb_sb[:], bc_ps[:])
                                        nc.vector.tensor_tensor(
                                            attnT[:, h,
                                                  qc * 512:(qc + 1) * 512],
                                            at_ps[:], rqb_sb[:],
                                            mybir.AluOpType.mult)
                                        if qc == QC - 1:
                                            # ship this head's attn slices
                                            for j in range(c.n_cores):
                                                eng = nc.sync if j % 2 == 0 \
                                                    else nc.scalar
                                                eng.dma_start(
                                                    a2a_in_rs[b][j][:, h],
                                                    attnT[:, h,
                                                          j * HB:
                                                          (j + 1) * HB])

                                    deferred.append(finalize)
                                if h == 0 and qc == QC - 1 and b + 1 < c.B:
                                    # prefetch next batch's first W group and
                                    # first X chunk (one ring slot each --
                                    # deeper prefetch would block the queue)
                                    pending_w[b + 1] = load_w(0)
                                    pending_x[b + 1] = \
                                        load_x1((b + 1) * c.S // CH, 0)
                            flush()
                        # per-batch AllToAll; b0's overlaps with b1 compute
                        nc.gpsimd.collective_compute(
                            "AllToAll",
                            mybir.AluOpType.bypass,
                            replica_groups=[list(range(c.n_cores))],
                            ins=[a2a_ins[b][:].opt()],
                            outs=[a2a_outs[b][:].opt()],
                        )

                # ---------------- Phase C: o_proj -------------------------
                with tc.tile_pool(name="catt", bufs=1) as cattp, \
                     tc.tile_pool(name="cwo", bufs=2) as wopool, \
                     tc.tile_pool(name="cout", bufs=4) as outpool, \
                     tc.tile_pool(name="cps", bufs=4, space="PSUM") as cpsum:
                    # my tokens: [0:HB) from batch 0, [HB:2*HB) from batch 1
                    att_sb = cattp.tile([128, KT, c.BLK], BF16, tag="catt")
                    gstep = max(1, KT // 4)
                    for bb in range(c.B):
                        for g0 in range(0, KT, gstep):
                            g1 = min(g0 + gstep, KT)
                            nc.sync.dma_start(
                                att_sb[:, g0:g1, bb * HB:(bb + 1) * HB],
                                a2a_out_rs[bb][:, g0:g1])
                    for oc in range(c.OC):
                        wo_sb = wopool.tile([128, KT, 512], BF16, tag="wo")
                        hk = KT // 2
                        nc.sync.dma_start(wo_sb[:, :hk], wog[:, oc, :hk])
                        nc.scalar.dma_start(wo_sb[:, hk:], wog[:, oc, hk:])
                        for tt in range(c.BLK // 128):
                            ps = cpsum.tile([128, 512], F32, tag="cps")
                            for ko in range(KT):
                                nc.tensor.matmul(
                                    ps[:],
                                    att_sb[:, ko, tt * 128:(tt + 1) * 128],
                                    wo_sb[:, ko],
                                    start=(ko == 0), stop=(ko == KT - 1))
                            o_sb = outpool.tile([128, 512], F32, tag="o")
                            nc.vector.tensor_copy(o_sb[:], ps[:])
                            nc.sync.dma_start(
                                out_ext[tt * 128:(tt + 1) * 128,
                                        oc * 512:(oc + 1) * 512],
                                o_sb[:])

    nc.compile()
    return nc


# --------------------------------------------------------------------------
_CACHE = {}


def _get_program(cfg: Cfg, mode: str):
    key = (cfg.key(), mode)
    if key not in _CACHE:
        _CACHE[key] = build_program(cfg, mode)
    return _CACHE[key]


def prepare_inputs(cfg: Cfg, hidden_states, attention_mask, W_pack, W_o):
    """Host-side shard + layout prep. Returns (mode, in_maps)."""
    c = cfg
    X = np.asarray(hidden_states, dtype=np.float32).reshape(c.T, c.hidden)
    # chunked X^T: [128, T/CH, KT, CH] (16KB-contiguous per-partition lines)
    CH = min(1024, c.S)
    XG = np.ascontiguousarray(
        X.reshape(c.T // CH, CH, c.KT, 128).transpose(3, 0, 2, 1)
    ).astype(NPBF16)

    mask = np.asarray(attention_mask, dtype=np.float32).reshape(c.S, c.S)
    causal_ref = np.where(
        np.tril(np.ones((c.S, c.S), dtype=bool)), 0.0, -1e9
    ).astype(np.float32)
    if np.array_equal(mask, causal_ref):
        mode = "causal"
    elif not mask.any():
        mode = "dense"
    else:
        mode = "masked"

    W_pack = np.asarray(W_pack, dtype=np.float32)
    W_o = np.asarray(W_o, dtype=np.float32)
    H, KT, OC = c.hidden, c.KT, c.OC
    # full W_o^T grouped for phase C: [128, OC, KT, 512]
    wog = np.ascontiguousarray(
        W_o.T.reshape(KT, 128, OC, 512).transpose(1, 2, 0, 3)).astype(NPBF16)
    maskT = None
    if mode == "masked":
        maskT = np.ascontiguousarray(mask.T * math.sqrt(c.dh),
                                     dtype=np.float32)
    NG = c.FT // 3
    in_maps = []
    for g in range(c.n_cores):
        r0, r1 = g * c.FO, (g + 1) * c.FO
        wq = W_pack[r0:r1]
        wk = W_pack[H + r0:H + r1]
        wv = W_pack[2 * H + r0:2 * H + r1]
        wqkvT = np.concatenate([wq, wk, wv], axis=0).T   # [H, F]
        # grouped for phase A: [128, NG, KT, 384]
        wgg = np.ascontiguousarray(
            wqkvT.reshape(KT, 128, NG, 384).transpose(1, 2, 0, 3)
        ).astype(NPBF16)
        m = {"xg": XG, "wg": wgg, "wog": wog}
        if mode == "masked":
            m["maskt"] = maskT
        in_maps.append(m)
    return mode, in_maps


def assemble_output(cfg: Cfg, results):
    c = cfg
    HB = c.S // c.n_cores
    full = np.empty((c.T, c.hidden), dtype=np.float32)
    for g in range(c.n_cores):
        o = results[g]["out"]
        for b in range(c.B):
            full[b * c.S + g * HB:b * c.S + (g + 1) * HB] = \
                o[b * HB:(b + 1) * HB]
    return full.reshape(c.B, c.S, c.hidden)


def kernel(hidden_states, attention_mask, W_pack, W_o):
    cfg = Cfg()
    mode, in_maps = prepare_inputs(cfg, hidden_states, attention_mask,
                                   W_pack, W_o)
    nc = _get_program(cfg, mode)
    res = bass_utils.run_bass_kernel_spmd(nc, in_maps,
                                          list(range(cfg.n_cores)))
    return assemble_output(cfg, res.results)


# revision 43
# speedup vs baseline: 1.7518x; 1.0476x over previous
"""Trainium2 Bass kernel for BaichuanAttention (hidden=5120, 40 heads, b=2, s=2048).

Tensor-parallel over heads across 8 NeuronCores, all-bf16 datapath:

  Phase A: per-core QKV projection (5 heads) in bf16, qkv kept resident in
           SBUF (no DRAM round-trip).
  Phase B: flash-style causal attention computing scores TRANSPOSED
           (S^T = K_tile^T Q) so exp() output lands directly in the
           [k, q] layout attn@V needs -- no PE/DVE transposes of P.
           Row-sums via a ones-vector matmul; V transposed to token-major
           with a single XBAR DMA-transpose per head.
  Phase C: AllToAll of the (normalized) per-head attention outputs
           ([T, 640] bf16 per core, 16x less wire than reduce-scattering
           o_proj partials), then each core runs o_proj for its own
           512-token slice against the full W_o.
"""

import math
import sys

for _p in ("/opt/trn_rl_repo",):
    if _p not in sys.path:
        sys.path.insert(0, _p)

import numpy as np
import ml_dtypes

import concourse.bass as bass
import concourse.mybir as mybir
import concourse.tile as tile
from concourse import bacc, bass_utils

F32 = mybir.dt.float32
BF16 = mybir.dt.bfloat16
NPBF16 = ml_dtypes.bfloat16


class Cfg:
    def __init__(self, hidden=5120, n_heads=40, dh=128, B=2, S=2048, n_cores=8):
        self.hidden = hidden
        self.n_heads = n_heads
        self.dh = dh
        self.B = B
        self.S = S
        self.n_cores = n_cores
        assert dh == 128
        self.HL = n_heads // n_cores          # heads per core (5)
        self.F = 3 * self.HL * dh             # per-core packed qkv rows (1920)
        self.FO = self.HL * dh                # per-core attn feature width (640)
        self.T = B * S                        # total tokens (4096)
        self.KT = hidden // 128               # contraction tiles for qkv (40)
        self.FT = self.F // 128               # qkv feature tiles (15)
        self.SQT = S // 128                   # seq 128-tiles per batch (16)
        self.QC = S // 512                    # q chunks per batch (4)
        self.OC = hidden // 512               # o_proj output chunks (10)
        self.BLK = self.T // n_cores          # tokens per core after A2A (512)

    def key(self):
        return (self.hidden, self.n_heads, self.dh, self.B, self.S, self.n_cores)


def build_program(cfg: Cfg, mode: str):
    """mode: 'causal' (mask input ignored, causal skip), 'dense' (zero mask),
    'masked' (general additive mask, host passes mask^T * sqrt(dh))."""
    assert mode in ("causal", "dense", "masked")
    c = cfg
    nc = bacc.Bacc("TRN2", target_bir_lowering=False, debug=False,
                   num_devices=c.n_cores)
    GW = 5 if c.FT % 5 == 0 else 3            # phase A ft-group width
    NG = c.FT // GW
    assert c.FT % GW == 0
    CH0 = 512
    # X^T chunked on host: [128, T/CH, KT, CH] -> 16KB-contiguous DMA lines
    xg_t = nc.dram_tensor("xg", [128, c.T // CH0, c.KT, CH0], BF16,
                          kind="ExternalInput").ap()
    # W_pack shard, pre-grouped on host: [128, NG, KT, GW*128] so each group
    # half is one fully-contiguous-per-partition DMA
    wg = nc.dram_tensor("wg", [128, NG, c.KT, GW * 128], BF16,
                        kind="ExternalInput").ap()
    # full W_o^T, pre-grouped: [128, OC, KT, 512]
    wog = nc.dram_tensor("wog", [128, c.OC, c.KT, 512], BF16,
                         kind="ExternalInput").ap()
    maskt = None
    if mode == "masked":
        maskt = nc.dram_tensor("maskt", [c.S, c.S], F32,
                               kind="ExternalInput").ap()
    out_ext = nc.dram_tensor("out", [c.BLK, c.hidden], F32,
                             kind="ExternalOutput").ap()

    inv_sqrt_dh = 1.0 / math.sqrt(c.dh)
    NEG = -1.0e9
    HL, KT, FT, QC, SQT = c.HL, c.KT, c.FT, c.QC, c.SQT

    maskt_r = maskt.rearrange("(kt p) q -> p kt q", p=128) if maskt is not None else None
    CH = CH0                                  # phase A token chunk
    NH2 = CH // 512

    # per-batch A2A: each batch's attn output is exchanged separately so the
    # collective overlaps with the other batch's compute. Rank i owns token
    # rows [i*HB, (i+1)*HB) of each batch (HB = S / n_cores).
    HB = c.S // c.n_cores
    with tile.TileContext(nc) as tc:
        with tc.tile_pool(name="dram", bufs=1, space="DRAM") as dram:
            a2a_ins = [dram.tile([c.n_cores, c.FO, HB], BF16,
                                 tag=f"a2ai{b}", name=f"a2ai{b}")
                       for b in range(c.B)]
            a2a_outs = [dram.tile([c.n_cores, c.FO, HB], BF16,
                                  tag=f"a2ao{b}", name=f"a2ao{b}")
                        for b in range(c.B)]
            a2a_in_rs = [t.rearrange("blk (h p) t -> blk p h t", p=128)
                         for t in a2a_ins]
            a2a_out_rs = [t.rearrange("g (f p) t -> p (g f) t", p=128)
                          for t in a2a_outs]

            with tc.tile_pool(name="const", bufs=1) as cpool:
                ones_col = cpool.tile([128, 1], BF16, tag="onec")
                ones_row = cpool.tile([1, 128], BF16, tag="oner")
                with tc.tile_pool(name="ctmp", bufs=1) as tmp:
                    o32a = tmp.tile([128, 1], F32, tag="o32a")
                    nc.vector.memset(o32a[:], 1.0)
                    nc.vector.tensor_copy(ones_col[:], o32a[:])
                    o32b = tmp.tile([1, 128], F32, tag="o32b")
                    nc.vector.memset(o32b[:], 1.0)
                    nc.vector.tensor_copy(ones_row[:], o32b[:])
                cmask = None
                if mode == "causal":
                    # triangular 128x128 mask: 0 where q >= k, else NEG
                    cmask = cpool.tile([128, 128], F32, tag="cmask")
                    nc.gpsimd.memset(cmask[:], 0.0)
                    nc.gpsimd.affine_select(
                        out=cmask[:], in_=cmask[:],
                        compare_op=mybir.AluOpType.is_ge, fill=NEG,
                        base=0, pattern=[[1, 128]],
                        channel_multiplier=-1)

                with tc.tile_pool(name="qkvp", bufs=1) as qkvpool, \
                     tc.tile_pool(name="attp", bufs=1) as attpool, \
                     tc.tile_pool(name="vtkp", bufs=2) as vpool, \
                     tc.tile_pool(name="aw", bufs=2) as wpool, \
                     tc.tile_pool(name="ax", bufs=2) as xpool:
                    pending_w = {}
                    pending_x = {}
                    HKT = KT // 2

                    def load_w_half(g, half):
                        # [128, KT/2, GW*128] ko-half of one ft-group, split
                        # across both DMA queues
                        w_sb = wpool.tile([128, HKT, GW * 128], BF16, tag="w")
                        k0 = half * HKT
                        qk = HKT // 2
                        nc.sync.dma_start(w_sb[:, :qk], wg[:, g, k0:k0 + qk])
                        nc.scalar.dma_start(w_sb[:, qk:],
                                            wg[:, g, k0 + qk:k0 + HKT])
                        return w_sb

                    def load_x1(ci_g, kb):
                        # one [128, 8, CH] chunk, split across both DMA queues
                        x_sb = xpool.tile([128, 8, CH], BF16, tag="x",
                                          bufs=3)
                        nc.sync.dma_start(
                            x_sb[:, :4], xg_t[:, ci_g, kb * 8:kb * 8 + 4])
                        nc.scalar.dma_start(
                            x_sb[:, 4:], xg_t[:, ci_g, kb * 8 + 4:kb * 8 + 8])
                        return x_sb

                    for b in range(c.B):
                        qkv_sb = qkvpool.tile([128, FT, c.S], BF16, tag="qkv",
                                              name=f"qkv{b}")
                        attnT = attpool.tile([128, HL, c.S], BF16, tag="att",
                                             name=f"att{b}")
                        # ---------------- Phase A: QKV projection ---------
                        with tc.tile_pool(name=f"aps{b}", bufs=1,
                                          space="PSUM") as apsum:
                            for g in range(NG):
                                ft0 = g * GW
                                whs = pending_w.pop(b, None) if g == 0 \
                                    else None
                                if whs is None:
                                    whs = [load_w_half(g, 0),
                                           load_w_half(g, 1)]
                                for ci in range(c.S // CH):
                                    ci_g = (b * c.S + ci * CH) // CH
                                    pss = [apsum.tile([128, 512], F32,
                                                      tag=f"aps{u}",
                                                      name=f"aps{u}")
                                           for u in range(GW * NH2)]
                                    for kb in range(KT // 8):
                                        x_sb = None
                                        if g == 0 and ci == 0 and kb == 0:
                                            x_sb = pending_x.pop(b, None)
                                        if x_sb is None:
                                            x_sb = load_x1(ci_g, kb)
                                        for kj in range(8):
                                            ko = kb * 8 + kj
                                            w_sb = whs[ko // HKT]
                                            kl = ko % HKT
                                            for i in range(GW):
                                                for hf in range(NH2):
                                                    nc.tensor.matmul(
                                                        pss[i * NH2 + hf][:],
                                                        w_sb[:, kl,
                                                             i * 128:
                                                             (i + 1) * 128],
                                                        x_sb[:, kj,
                                                             hf * 512:
                                                             (hf + 1) * 512],
                                                        start=(ko == 0),
                                                        stop=(ko == KT - 1))
                                    for i in range(GW):
                                        for hf in range(NH2):
                                            o0 = ci * CH + hf * 512
                                            nc.vector.tensor_copy(
                                                qkv_sb[:, ft0 + i,
                                                       o0:o0 + 512],
                                                pss[i * NH2 + hf][:])

                        # ---------------- Phase B: attention --------------
                        with tc.tile_pool(name=f"bs{b}", bufs=2,
                                          space="PSUM") as spool, \
                             tc.tile_pool(name=f"bat{b}", bufs=2,
                                          space="PSUM") as batp, \
                             tc.tile_pool(name=f"brs{b}", bufs=2,
                                          space="PSUM") as rsbc, \
                             tc.tile_pool(name=f"bp{b}", bufs=9) as ppool, \
                             tc.tile_pool(name=f"bm{b}", bufs=3) as mpool, \
                             tc.tile_pool(name=f"bsm{b}", bufs=2) as smpool:
                            deferred = []

                            def flush():
                                while deferred:
                                    deferred.pop(0)()

                            for h in range(HL):
                                v_tok = vpool.tile([128, SQT, 128], BF16,
                                                   tag="vtok")
                                nc.sync.dma_start_transpose(
                                    v_tok[:], qkv_sb[:, 2 * HL + h, :])
                                for qc in range(QC):
                                    nkp = 2 * (qc + 1) if mode == "causal" \
                                        else SQT // 2
                                    nkt = 2 * nkp
                                    rs_ps = rsbc.tile([1, 512], F32,
                                                      tag="rsbc", name="rs")
                                    at_ps = batp.tile([128, 512], F32,
                                                      tag="at")
                                    pts = []

                                    def lo_of(kt):
                                        # first valid q column of k-tile kt
                                        if mode != "causal":
                                            return 0
                                        return max(0, kt * 128 - qc * 512)

                                    def emit_one_at(j, rs_ps=rs_ps,
                                                    at_ps=at_ps, pts=pts,
                                                    v_tok=v_tok, nkt=nkt,
                                                    lo_of=lo_of):
                                        for half in range(2):
                                            kt = 2 * j + half
                                            lo = lo_of(kt)
                                            nc.tensor.matmul(
                                                rs_ps[:, lo:], ones_col[:],
                                                pts[j][:, half, lo:],
                                                start=(kt == 0),
                                                stop=(kt == nkt - 1))
                                            nc.tensor.matmul(
                                                at_ps[:, lo:], v_tok[:, kt],
                                                pts[j][:, half, lo:],
                                                start=(kt == 0),
                                                stop=(kt == nkt - 1))

                                    for kp in range(nkp):
                                        s_ps = spool.tile([128, 2, 512], F32,
                                                          tag="s")
                                        for half in range(2):
                                            kt = 2 * kp + half
                                            lo = lo_of(kt)
                                            nc.tensor.matmul(
                                                s_ps[:, half],
                                                qkv_sb[:, HL + h,
                                                       kt * 128:(kt + 1) * 128],
                                                qkv_sb[:, h,
                                                       qc * 512:(qc + 1) * 512],
                                                start=True, stop=True)
                                            if mode == "causal" \
                                                    and kt >= 4 * qc:
                                                # diagonal: triangular mask on
                                                # q columns [lo, lo+128)
                                                nc.vector.tensor_tensor(
                                                    s_ps[:, half,
                                                         lo:lo + 128],
                                                    s_ps[:, half,
                                                         lo:lo + 128],
                                                    cmask[:],
                                                    mybir.AluOpType.add)
                                        if mode == "masked":
                                            m_sb = mpool.tile([128, 2, 512],
                                                              F32, tag="m")
                                            nc.sync.dma_start(
                                                m_sb[:],
                                                maskt_r[:, 2 * kp:2 * kp + 2,
                                                        qc * 512:(qc + 1) * 512])
                                            nc.vector.tensor_tensor(
                                                s_ps[:], s_ps[:], m_sb[:],
                                                mybir.AluOpType.add)
                                        p_sb = ppool.tile([128, 2, 512], BF16,
                                                          tag="p")
                                        nc.scalar.activation(
                                            p_sb[:], s_ps[:],
                                            mybir.ActivationFunctionType.Exp,
                                            scale=inv_sqrt_dh)
                                        pts.append(p_sb)
                                        if kp == 1:
                                            flush()
                                        if kp >= 2:
                                            emit_one_at(kp - 2)
                                    if nkp == 1:
                                        flush()
                                    emit_one_at(nkp - 2)
                                    emit_one_at(nkp - 1)

                                    def finalize(h=h, qc=qc, rs_ps=rs_ps,
                                                 at_ps=at_ps, attnT=attnT,
                                                 b=b):
                                        # broadcast the row-sums to all 128
                                        # partitions first, then reciprocal
                                        # runs on 128 lanes instead of 1
                                        rsbf = smpool.tile([1, 512], BF16,
                                                           tag="rsbf")
                                        nc.scalar.copy(rsbf[:], rs_ps[:])
                                        bc_ps = rsbc.tile([128, 512], F32,
                                                          tag="rsbc",
                                                          name="bc")
                                        nc.tensor.matmul(bc_ps[:], ones_row[:],
                                                         rsbf[:], start=True,
                                                         stop=True)
                                        rqb_sb = smpool.tile([128, 512], F32,
                                                             tag="rqb")
                                        nc.vector.reciprocal_approx_fast(
                                            rqb_sb[:], bc_ps[:])
                                        nc.vector.tensor_tensor(
                                            attnT[:, h,
                                                  qc * 512:(qc + 1) * 512],
                                            at_ps[:], rqb_sb[:],
                                            mybir.AluOpType.mult)
                                        if qc == QC - 1:
                                            # ship this head's attn slices
                                            for j in range(c.n_cores):
                                                eng = nc.sync if j % 2 == 0 \
                                                    else nc.scalar
                                                eng.dma_start(
                                                    a2a_in_rs[b][j][:, h],
                                                    attnT[:, h,
                                                          j * HB:
                                                          (j + 1) * HB])

                                    deferred.append(finalize)
                                if h == 0 and qc == QC - 1 and b + 1 < c.B:
                                    # prefetch next batch's first W group and
                                    # first X chunk (within ring capacity --
                                    # deeper prefetch would block the queue)
                                    pending_w[b + 1] = [load_w_half(0, 0),
                                                        load_w_half(0, 1)]
                                    pending_x[b + 1] = \
                                        load_x1((b + 1) * c.S // CH, 0)
                            flush()
                        # per-batch AllToAll; b0's overlaps with b1 compute
                        nc.gpsimd.collective_compute(
                            "AllToAll",
                            mybir.AluOpType.bypass,
                            replica_groups=[list(range(c.n_cores))],
                            ins=[a2a_ins[b][:].opt()],
                            outs=[a2a_outs[b][:].opt()],
                        )

                # ---------------- Phase C: o_proj -------------------------
                with tc.tile_pool(name="catt", bufs=1) as cattp, \
                     tc.tile_pool(name="cwo", bufs=3) as wopool, \
                     tc.tile_pool(name="cout", bufs=4) as outpool, \
                     tc.tile_pool(name="cps", bufs=4, space="PSUM") as cpsum:
                    # my tokens: [0:HB) from batch 0, [HB:2*HB) from batch 1.
                    # Emit the first PRE ocs' batch-0 token-tiles before any
                    # batch-1 work so the second A2A hides under them; the
                    # W_o stream for those ocs is shared (bufs=PRE ring).
                    att_sb = cattp.tile([128, KT, c.BLK], BF16, tag="catt")
                    gstep = max(1, KT // 4)
                    TT = c.BLK // 128
                    h2 = TT // 2

                    def load_att(bb):
                        for g0 in range(0, KT, gstep):
                            g1 = min(g0 + gstep, KT)
                            eng = nc.sync if g0 % (2 * gstep) == 0 \
                                else nc.scalar
                            eng.dma_start(
                                att_sb[:, g0:g1, bb * HB:(bb + 1) * HB],
                                a2a_out_rs[bb][:, g0:g1])

                    def load_wo(oc):
                        wo_sb = wopool.tile([128, KT, 512], BF16, tag="wo")
                        hk = KT // 2
                        nc.sync.dma_start(wo_sb[:, :hk], wog[:, oc, :hk])
                        nc.scalar.dma_start(wo_sb[:, hk:], wog[:, oc, hk:])
                        return wo_sb

                    def emit_oc(oc, wo_sb, tts):
                        for tt in tts:
                            ps = cpsum.tile([128, 512], F32, tag="cps")
                            for ko in range(KT):
                                nc.tensor.matmul(
                                    ps[:],
                                    att_sb[:, ko, tt * 128:(tt + 1) * 128],
                                    wo_sb[:, ko],
                                    start=(ko == 0), stop=(ko == KT - 1))
                            o_sb = outpool.tile([128, 512], F32, tag="o")
                            nc.vector.tensor_copy(o_sb[:], ps[:])
                            nc.sync.dma_start(
                                out_ext[tt * 128:(tt + 1) * 128,
                                        oc * 512:(oc + 1) * 512],
                                o_sb[:])

                    PRE = min(3, c.OC) if c.B > 1 else 0
                    load_att(0)
                    wos = []
                    for oc in range(PRE):
                        wos.append(load_wo(oc))
                        emit_oc(oc, wos[oc], range(h2))
                    if c.B > 1:
                        load_att(1)
                    for oc in range(PRE):
                        emit_oc(oc, wos[oc], range(h2, TT))
                    for oc in range(PRE, c.OC):
                        wo_sb = load_wo(oc)
                        emit_oc(oc, wo_sb, range(TT))

    nc.compile()
    return nc


# --------------------------------------------------------------------------
_CACHE = {}


def _get_program(cfg: Cfg, mode: str):
    key = (cfg.key(), mode)
    if key not in _CACHE:
        _CACHE[key] = build_program(cfg, mode)
    return _CACHE[key]


def prepare_inputs(cfg: Cfg, hidden_states, attention_mask, W_pack, W_o):
    """Host-side shard + layout prep. Returns (mode, in_maps)."""
    c = cfg
    X = np.asarray(hidden_states, dtype=np.float32).reshape(c.T, c.hidden)
    # chunked X^T: [128, T/CH, KT, CH] (contiguous per-partition lines)
    CH = 512
    XG = np.ascontiguousarray(
        X.reshape(c.T // CH, CH, c.KT, 128).transpose(3, 0, 2, 1)
    ).astype(NPBF16)

    mask = np.asarray(attention_mask, dtype=np.float32).reshape(c.S, c.S)
    causal_ref = np.where(
        np.tril(np.ones((c.S, c.S), dtype=bool)), 0.0, -1e9
    ).astype(np.float32)
    if np.array_equal(mask, causal_ref):
        mode = "causal"
    elif not mask.any():
        mode = "dense"
    else:
        mode = "masked"

    W_pack = np.asarray(W_pack, dtype=np.float32)
    W_o = np.asarray(W_o, dtype=np.float32)
    H, KT, OC = c.hidden, c.KT, c.OC
    # full W_o^T grouped for phase C: [128, OC, KT, 512]
    wog = np.ascontiguousarray(
        W_o.T.reshape(KT, 128, OC, 512).transpose(1, 2, 0, 3)).astype(NPBF16)
    maskT = None
    if mode == "masked":
        maskT = np.ascontiguousarray(mask.T * math.sqrt(c.dh),
                                     dtype=np.float32)
    GW = 5 if c.FT % 5 == 0 else 3
    NG = c.FT // GW
    in_maps = []
    for g in range(c.n_cores):
        r0, r1 = g * c.FO, (g + 1) * c.FO
        wq = W_pack[r0:r1]
        wk = W_pack[H + r0:H + r1]
        wv = W_pack[2 * H + r0:2 * H + r1]
        wqkvT = np.concatenate([wq, wk, wv], axis=0).T   # [H, F]
        # grouped for phase A: [128, NG, KT, GW*128]
        wgg = np.ascontiguousarray(
            wqkvT.reshape(KT, 128, NG, GW * 128).transpose(1, 2, 0, 3)
        ).astype(NPBF16)
        m = {"xg": XG, "wg": wgg, "wog": wog}
        if mode == "masked":
            m["maskt"] = maskT
        in_maps.append(m)
    return mode, in_maps


def assemble_output(cfg: Cfg, results):
    c = cfg
    HB = c.S // c.n_cores
    full = np.empty((c.T, c.hidden), dtype=np.float32)
    for g in range(c.n_cores):
        o = results[g]["out"]
        for b in range(c.B):
            full[b * c.S + g * HB:b * c.S + (g + 1) * HB] = \
                o[b * HB:(b + 1) * HB]
    return full.reshape(c.B, c.S, c.hidden)


def kernel(hidden_states, attention_mask, W_pack, W_o):
    cfg = Cfg()
    mode, in_maps = prepare_inputs(cfg, hidden_states, attention_mask,
                                   W_pack, W_o)
    nc = _get_program(cfg, mode)
    res = bass_utils.run_bass_kernel_spmd(nc, in_maps,
                                          list(range(cfg.n_cores)))
    return assemble_output(cfg, res.results)
